# revision 14
# baseline (speedup 1.0000x reference)
"""AdaIN statistics kernel for TRN2, SPMD across 8 NeuronCores. v4.

Input : f_vol [32, 512, 64, 64] f32
Output: [32, 1024] f32 = concat([mean over (h,w), unbiased std over (h,w)], axis=-1)

Sharding: data-parallel over batch - each of the 8 cores handles 4 batches
([4, 512, 64, 64] shard, 32 MiB). No collectives; the host concatenates the
8 per-core [4, 1024] outputs.

v4 design, from trace evidence of v2 (uniform interleaved deal, per-slab
scattered out-DMAs) and v3 (partition-rebalanced deal):
  - Input DMAs must be uniform 128-partition transfers with large (32 KiB)
    per-partition descriptors: those ran at full per-engine line rate
    (~26.7 B/ns) on ALL 16 queue rings in v3.  Sub-128-partition DMAs get
    concentrated onto engine quads (v3's asym slabs overloaded engines
    0-3 2.5x) and small descriptors run at half rate.
  - v2's outputs were ~1800 scattered 8-byte descriptors (per-slab
    [[2,P],[C,2],[1,2]] APs); v2 showed one ring running ~21% slow with
    exactly the per-slab periodicity of those outs.  v4 eliminates them.

Lane-major deal: lane p owns rows 16p..16p+15 (row = b*512+c of the
[2048, 4096] row-major view).  Slab s (s=0..7) moves rows {16p+2s,
16p+2s+1}: per-partition 32 KiB contiguous, one clean DMA.  Outputs:
means/stds accumulate on-chip into MS[128, 2, 16] (lane p, mean|std,
slot); because lane p's rows are 16 consecutive c-positions of batch
p//32, the whole output is expressible as TWO DMAs:
    dst AP [[1024, 4], [16, 32], [512, 2], [1, w]]  (2-level partition dim)
an early one for slots 0..13 (hidden mid-stream) and a final one for
slots 14..15.

The last slab (s=7) is column-chunked (2048/1024/512/384/128 cols) so DVE
pipelines bn_stats against chunk arrivals; only the final 128-col chunk's
2 bn_stats + 2 bn_aggr + 4 ACT ops + 1 tiny out-DMA trail the last byte.

Compute split: ACT consumes slabs s1,s2,s3 via Copy/Square+accumulate
(freeing DVE to keep up with the stream); DVE does bn_stats/bn_aggr for
s0, s4, s5, s6, s7.  Ring: 5 xt slots; s5->slot0 (after s0 consumed by
DVE), s6->slot1 (after s1: ACT), s7->slot2 (after s2: ACT).

Hard-won semaphore lesson (v2): same-engine RAW through SBUF is NOT
covered by program order.  Every producer->consumer edge carries an
explicit semaphore observation.  SWDGE requires sem values to start at 0,
so semaphores are never reused; stats/mv/res buffers are never reused.
"""

from contextlib import ExitStack

import numpy as np

B, C, H, W = 32, 512, 64, 64
N_CORES = 8
B_LOCAL = B // N_CORES  # 4
N = H * W  # 4096
P = 128
ROWS = B_LOCAL * C  # 2048
RPL = ROWS // P  # 16 rows per lane

NBUF = 5
NSLAB = 8  # slabs of 2 rows/lane
CHUNKS = [2048, 1024, 512, 384, 128]  # column chunks of the last slab
assert sum(CHUNKS) == N
# bn_stats group widths per row for the chunked slab (FMAX=512)
GROUPS7 = [512] * 7 + [384, 128]
DVE_SLABS = (0, 4, 5, 6)  # full slabs on DVE (s7 chunked, also DVE)
ACT_SLABS = (1, 2, 3)  # slabs consumed by ACT accumulate
EARLY_SLOTS = 14  # slots 0..13 in the early out-DMA; 14..15 in the final
# Ring-15 rebalance: DMAs with n<=16 descriptors land on engines 0..n-1
# (verified: n=4 -> engines 0-3, n=12 -> 0-11, n=32 -> 2/engine).  Slabs in
# SPLIT_SLABS are issued as 8 pieces of 15 partitions + one of 8, so queue
# ring 15 (the systematically ~21%-slow engine) gets no descriptors from
# them: its load drops 64 -> 48 slab-descs (~73 us) while engines 0-7 rise
# to ~66 (~82 us), levelling the stream end.
SPLIT_SLABS = (1, 4)
SPLIT_PIECES = [(15 * i, 15) for i in range(8)] + [(120, 8)]

_CACHE = {}


def _build():
    import concourse.bass as bass
    from concourse import mybir

    nc = bass.Bass()
    x_ext = nc.declare_dram_parameter(
        "f_vol", [B_LOCAL, C, H, W], mybir.dt.float32, isOutput=False
    )
    out_ext = nc.declare_dram_parameter(
        "out", [B_LOCAL, 2 * C], mybir.dt.float32, isOutput=True
    )

    # [128, 16*4096]: lane p <- rows 16p..16p+15, contiguous per lane
    xl = (
        x_ext.ap()
        .rearrange("b c h w -> (b c) (h w)")
        .rearrange("(p u) f -> p (u f)", u=RPL)
    )

    # semaphore plans (cumulative, by emission order)
    # dve_stats: s0 (16), s4 (16), s5, s6, then chunks of s7 (2 per chunk)
    dve_cum = {0: 16, 4: 32, 5: 48, 6: 64}
    DVE_TOTAL = 64 + 2 * len(CHUNKS)
    # mv_ready: 2 per DVE slab in the same order, +2 for s7
    mv_after = {0: 2, 4: 4, 5: 6, 6: 8, 7: 10}
    # act_stats: +1 per ACT accumulate pass; s1, s2, s3 get 4 each
    acts_after = {1: 4, 2: 8, 3: 12}

    with ExitStack() as ctx:
        block = ctx.enter_context(nc.Block(no_gpsimd_drain=True))
        dma_s = [ctx.enter_context(nc.semaphore(f"dma_s{s}")) for s in range(7)]
        dma_c = [
            ctx.enter_context(nc.semaphore(f"dma_c{g}")) for g in range(len(CHUNKS))
        ]
        out_sem = ctx.enter_context(nc.semaphore("out_sem"))
        dve_stats = ctx.enter_context(nc.semaphore("dve_stats"))
        mv_ready = ctx.enter_context(nc.semaphore("mv_ready"))
        act_stats = ctx.enter_context(nc.semaphore("act_stats"))
        act_done = ctx.enter_context(nc.semaphore("act_done"))
        warm_done = ctx.enter_context(nc.semaphore("warm_done"))

        xt = ctx.enter_context(nc.sbuf_tensor("xt", [P, NBUF, 2 * N], mybir.dt.float32))
        # stats slot per DVE slab: 0,4,5,6 -> 0..3; s7 -> 4 (9 groups)
        stats = ctx.enter_context(
            nc.sbuf_tensor("stats", [P, 5, 2, 9, 6], mybir.dt.float32)
        )
        mv = ctx.enter_context(nc.sbuf_tensor("mv", [P, 5, 2, 2], mybir.dt.float32))
        # output image: [lane, mean|std, slot]
        MS = ctx.enter_context(nc.sbuf_tensor("MS", [P, 2, RPL], mybir.dt.float32))
        acc = ctx.enter_context(nc.sbuf_tensor("acc", [P, 3, 2, 3], mybir.dt.float32))
        warm = ctx.enter_context(nc.sbuf_tensor("warm", [P, 2], mybir.dt.float32))

        sslot = {0: 0, 4: 1, 5: 2, 6: 3, 7: 4}  # stats/mv slot per DVE slab
        ring = {s: s % NBUF for s in range(NSLAB)}  # s5->0, s6->1, s7->2

        # act_done cumulative gates, in ACT emission order:
        # epi(s0)=4 mv-form; epi(s1..s3)=8 acc-form; epi(s4..s6)=4; epi(s7)=4
        actd = {}
        cact_plan = 0
        for s, n in [(0, 4), (1, 8), (2, 8), (3, 8), (4, 4), (5, 4), (6, 4), (7, 4)]:
            cact_plan += n
            actd[s] = cact_plan
        ACT_TOTAL = cact_plan

        def slab_src(s):
            return xl[:, 2 * s * N : (2 * s + 2) * N]

        @block.sync
        def _(sync):
            for s in range(7):
                if s >= NBUF:
                    sp = s - NBUF
                    if sp in ACT_SLABS:
                        sync.wait_ge(act_stats, acts_after[sp])
                    else:
                        sync.wait_ge(dve_stats, dve_cum[sp])
                src = slab_src(s)
                if s in SPLIT_SLABS:
                    for p0, np_ in SPLIT_PIECES:
                        sync.dma_start(
                            out=xt[p0 : p0 + np_, ring[s], :],
                            in_=src[p0 : p0 + np_, :],
                        ).then_inc(dma_s[s], 16)
                else:
                    sync.dma_start(out=xt[:, ring[s], :], in_=src).then_inc(
                        dma_s[s], 16
                    )
            # s7: column-chunked into ring slot 2 (s2 is ACT-consumed)
            sync.wait_ge(act_stats, acts_after[2])
            xs7 = slab_src(7).rearrange("p (m f) -> p m f", f=N)
            xd7 = xt[:, ring[7], :].rearrange("p (m f) -> p m f", f=N)
            c0 = 0
            for g, w in enumerate(CHUNKS):
                sync.dma_start(
                    out=xd7[:, :, c0 : c0 + w], in_=xs7[:, :, c0 : c0 + w]
                ).then_inc(dma_c[g], 16)
                c0 += w
            sync.wait_ge(out_sem, 64)

        @block.vector
        def _(vector):
            ndve = 0
            nmv = 0

            vector.memset(warm[:, :], 0.0).then_inc(warm_done, 1)

            for s in DVE_SLABS:
                ss = sslot[s]
                vector.wait_ge(dma_s[s], 16 * (9 if s in SPLIT_SLABS else 1))
                for r in range(2):
                    for g in range(8):
                        vector.bn_stats(
                            out=stats[:, ss, r, g, :],
                            in_=xt[:, ring[s], (r * 8 + g) * 512 : (r * 8 + g + 1) * 512],
                        ).then_inc(dve_stats, 1)
                        ndve += 1
                assert ndve == dve_cum[s]
                vector.wait_ge(dve_stats, ndve)
                for r in range(2):
                    vector.bn_aggr(
                        out=mv[:, ss, r, :], in_=stats[:, ss, r, 0:8, :]
                    ).then_inc(mv_ready, 1)
                    nmv += 1

            # s7 chunks: bn_stats pipelined against chunk arrivals
            ss = sslot[7]
            c0 = 0
            gi = 0
            for g, w in enumerate(CHUNKS):
                vector.wait_ge(dma_c[g], 16)
                # groups covered by this chunk: consecutive, widths from GROUPS7
                cend = c0 + w
                gg = c0
                while gg < cend:
                    gw = GROUPS7[gi]
                    for r in range(2):
                        vector.bn_stats(
                            out=stats[:, ss, r, gi, :],
                            in_=xt[:, ring[7], r * N + gg : r * N + gg + gw],
                        ).then_inc(dve_stats, 1)
                        ndve += 1
                    gg += gw
                    gi += 1
                assert gg == cend
                c0 = cend
            assert gi == len(GROUPS7)
            # GROUPS7 chunks each contain 2 bn_stats per 512-col group...
            vector.wait_ge(dve_stats, ndve)
            for r in range(2):
                vector.bn_aggr(
                    out=mv[:, ss, r, :], in_=stats[:, ss, r, :, :]
                ).then_inc(mv_ready, 1)
                nmv += 1
            assert nmv == mv_after[7]

        @block.scalar
        def _(scalar):
            A = 1.0 / np.sqrt(float(N) * (N - 1))
            cact = 0
            nacc = 0

            scalar.wait_ge(warm_done, 1)
            scalar.activation(
                out=warm[:, 0:1],
                in_=warm[:, 1:2],
                func=mybir.ActivationFunctionType.Copy,
            )

            def acc_pass(s):
                nonlocal nacc
                ai = s - 1  # acc slot for s in {1,2,3}
                scalar.wait_ge(dma_s[s], 16 * (9 if s in SPLIT_SLABS else 1))
                for r in range(2):
                    row = xt[:, ring[s], r * N : (r + 1) * N]
                    scalar.activation(
                        out=row,
                        in_=row,
                        func=mybir.ActivationFunctionType.Copy,
                        accum_out=acc[:, ai, r, 0:1],
                    ).then_inc(act_stats, 1)
                    nacc += 1
                    scalar.wait_ge(act_stats, nacc)
                    scalar.activation(
                        out=row,
                        in_=row,
                        func=mybir.ActivationFunctionType.Square,
                        accum_out=acc[:, ai, r, 1:2],
                    ).then_inc(act_stats, 1)
                    nacc += 1
                assert nacc == acts_after[s]

            def mv_stat_ops(mean_src, var_src, mean_dst, std_dst):
                nonlocal cact
                scalar.copy(out=mean_dst, in_=mean_src).then_inc(act_done, 1)
                scalar.activation(
                    out=std_dst,
                    in_=var_src,
                    func=mybir.ActivationFunctionType.Sqrt,
                    scale=float(N) / (N - 1),
                ).then_inc(act_done, 1)
                cact += 2

            def acc_stat_ops(ai, r, mean_dst, std_dst):
                nonlocal cact
                scalar.activation(
                    out=mean_dst,
                    in_=acc[:, ai, r, 0:1],
                    func=mybir.ActivationFunctionType.Copy,
                    scale=1.0 / N,
                ).then_inc(act_done, 1)
                scalar.activation(
                    out=acc[:, ai, r, 2:3],
                    in_=acc[:, ai, r, 0:1],
                    func=mybir.ActivationFunctionType.Square,
                    scale=A,
                ).then_inc(act_done, 1)
                cact += 2
                scalar.wait_ge(act_done, cact)
                scalar.activation(
                    out=acc[:, ai, r, 2:3],
                    in_=acc[:, ai, r, 2:3],
                    func=mybir.ActivationFunctionType.Copy,
                    scale=-1.0,
                ).then_inc(act_done, 1)
                cact += 1
                scalar.wait_ge(act_done, cact)
                scalar.activation(
                    out=std_dst,
                    in_=acc[:, ai, r, 1:2],
                    func=mybir.ActivationFunctionType.Sqrt,
                    scale=1.0 / (N - 1),
                    bias=acc[:, ai, r, 2:3],
                ).then_inc(act_done, 1)
                cact += 1

            def epi(s):
                # results into MS[:, 0, 2s+r] (mean) and MS[:, 1, 2s+r] (std)
                if s in ACT_SLABS:
                    scalar.wait_ge(act_stats, acts_after[s])
                    for r in range(2):
                        acc_stat_ops(
                            s - 1,
                            r,
                            MS[:, 0, 2 * s + r : 2 * s + r + 1],
                            MS[:, 1, 2 * s + r : 2 * s + r + 1],
                        )
                else:
                    scalar.wait_ge(mv_ready, mv_after[s])
                    ss = sslot[s]
                    for r in range(2):
                        mv_stat_ops(
                            mv[:, ss, r, 0:1],
                            mv[:, ss, r, 1:2],
                            MS[:, 0, 2 * s + r : 2 * s + r + 1],
                            MS[:, 1, 2 * s + r : 2 * s + r + 1],
                        )
                assert cact == actd[s], (s, cact, actd[s])

            def out_dma(u0, w):
                # one DMA for means, one for stds (APs are limited to 3 dims)
                for is_std in range(2):
                    dst = bass.AP(
                        tensor=out_ext,
                        offset=is_std * C + u0,
                        ap=[[2 * C, B_LOCAL], [RPL, P // B_LOCAL], [1, w]],
                    )
                    scalar.dma_start(
                        out=dst, in_=MS[:, is_std, u0 : u0 + w]
                    ).then_inc(out_sem, 16)

            acc_pass(1)
            acc_pass(2)
            epi(0)
            acc_pass(3)
            epi(1)
            epi(2)
            epi(3)
            epi(4)
            epi(5)
            epi(6)
            # early out: slots 0..13 (slabs 0-6) - hidden mid-stream
            scalar.wait_ge(act_done, actd[6])
            out_dma(0, EARLY_SLOTS)
            epi(7)
            scalar.wait_ge(act_done, actd[7])
            out_dma(EARLY_SLOTS, RPL - EARLY_SLOTS)
            assert cact == ACT_TOTAL

    return nc


def kernel(f_vol: np.ndarray) -> np.ndarray:
    from concourse.bass_utils import run_bass_kernel_spmd

    if "nc" not in _CACHE:
        _CACHE["nc"] = _build()
    nc = _CACHE["nc"]

    f_vol = np.ascontiguousarray(f_vol, dtype=np.float32)
    in_maps = [
        {"f_vol": f_vol[i * B_LOCAL : (i + 1) * B_LOCAL]} for i in range(N_CORES)
    ]
    res = run_bass_kernel_spmd(nc, in_maps, core_ids=list(range(N_CORES)))
    return np.concatenate([res.results[i]["out"] for i in range(N_CORES)], axis=0)


# revision 17
# speedup vs baseline: 1.3527x; 1.3527x over previous
"""AdaIN statistics kernel for TRN2, SPMD across 8 NeuronCores. v6.

Input : f_vol [32, 512, 64, 64] f32
Output: [32, 1024] f32 = concat([mean over (h,w), unbiased std over (h,w)], axis=-1)

Sharding: data-parallel over batch - each of the 8 cores handles 4 batches
([4, 512, 64, 64] shard, 32 MiB). No collectives; the host concatenates the
8 per-core [4, 1024] outputs.

Hard-won DMA facts from v2-v5 traces:
  - Only uniform 128-partition DMAs with large (16-32 KiB) per-partition
    descriptors run at per-engine line rate (~26.7 B/ns).  Sub-128-
    partition DMAs are assigned positionally (desc i -> engine i for
    n<=16) AND halve the throughput of the whole queue (v5: 15-partition
    pieces dropped every engine to ~14.4 B/ns).  So every bulk transfer
    here is a full-128-lane slab.
  - Queue ring 15 runs ~21% slower than rings 0-14 when all rings are
    loaded (v2/v4; absent when underloaded, v3).  Its backlog sets the
    stream end and delays every completion semaphore.  v6 issues half the
    slabs on the second HWDGE queue (scalar/ACT ring) to probe whether
    the slowdown is per-ring-fetch (two rings pipeline) or per-engine.

Lane-major deal: lane p owns rows 16p..16p+15 (row = b*512+c of the
[2048, 4096] row-major view).  Slabs s0..s6 move 2 rows/lane (32 KiB
descriptors); slot 14 (s7a) is one 16 KiB-descriptor slab; slot 15 (s7b)
is one row split into 9 column chunks (7x512, 384, 128) so DVE pipelines
bn_stats against chunk arrivals and only the last 128-col chunk's
1 bn_stats + 1 bn_aggr + 2 ACT ops + 2 tiny out-DMAs trail the stream.

Outputs accumulate on-chip into MS[128, 2, 16] (lane, mean|std, slot);
lane p's rows are 16 consecutive c-positions of batch p//32, so the whole
output is 4 DMAs: early mean/std for slots 0..14 (hidden mid-stream) and
final mean/std for slot 15 (mean from scalar, std from sync, in parallel).

Compute split: ACT consumes slabs s1,s2,s3 via Copy/Square+accumulate;
DVE does bn_stats/bn_aggr for s0,s4,s5,s6,s7a,s7b.

Semaphore discipline (v2 lesson): same-engine RAW through SBUF is NOT
covered by program order; every producer->consumer edge carries an
explicit semaphore observation.  Semaphores are never reused; stats/mv
buffers are never reused.
"""

from contextlib import ExitStack

import numpy as np

B, C, H, W = 32, 512, 64, 64
N_CORES = 8
B_LOCAL = B // N_CORES  # 4
N = H * W  # 4096
P = 128
ROWS = B_LOCAL * C  # 2048
RPL = ROWS // P  # 16 rows per lane

CHUNKS = [512] * 7 + [384, 128]  # column chunks of the s7b row
assert sum(CHUNKS) == N
SYNC_SLABS = (0, 2, 5, 6)  # full slabs issued on the sync (SP) queue
SCAL_SLABS = (1, 3, 4)  # full slabs issued on the scalar (ACT) queue
ACT_SLABS = (1, 2, 3)  # slabs consumed by ACT accumulate
DVE_SLABS = (0, 4, 5, 6)  # full slabs on DVE (s7a/s7b also DVE)
RING = {0: 0, 1: 1, 2: 2, 3: 3, 4: 4, 5: 0, 6: 1, 7: 2, 8: 3}  # 7=s7a, 8=s7b

_CACHE = {}


def _build():
    import concourse.bass as bass
    from concourse import mybir

    nc = bass.Bass()
    x_ext = nc.declare_dram_parameter(
        "f_vol", [B_LOCAL, C, H, W], mybir.dt.float32, isOutput=False
    )
    out_ext = nc.declare_dram_parameter(
        "out", [B_LOCAL, 2 * C], mybir.dt.float32, isOutput=True
    )

    # [128, 16*4096]: lane p <- rows 16p..16p+15, contiguous per lane
    xl = (
        x_ext.ap()
        .rearrange("b c h w -> (b c) (h w)")
        .rearrange("(p u) f -> p (u f)", u=RPL)
    )

    # cumulative semaphore plans (emission order)
    dve_cum = {0: 16, 4: 32, 5: 48, 6: 64, 7: 72}  # + chunks -> 81
    mv_after = {0: 2, 4: 4, 5: 6, 6: 8, 7: 9, 8: 10}
    acts_after = {1: 4, 2: 8, 3: 12}

    with ExitStack() as ctx:
        block = ctx.enter_context(nc.Block(no_gpsimd_drain=True))
        dma_s = [ctx.enter_context(nc.semaphore(f"dma_s{s}")) for s in range(8)]
        dma_c = [
            ctx.enter_context(nc.semaphore(f"dma_c{g}")) for g in range(len(CHUNKS))
        ]
        out_sem = ctx.enter_context(nc.semaphore("out_sem"))
        fin_sem = ctx.enter_context(nc.semaphore("fin_sem"))
        dve_stats = ctx.enter_context(nc.semaphore("dve_stats"))
        mv_ready = ctx.enter_context(nc.semaphore("mv_ready"))
        act_stats = ctx.enter_context(nc.semaphore("act_stats"))
        act_done = ctx.enter_context(nc.semaphore("act_done"))
        warm_done = ctx.enter_context(nc.semaphore("warm_done"))

        xt = ctx.enter_context(nc.sbuf_tensor("xt", [P, 5, 2 * N], mybir.dt.float32))
        # stats slots: s0,s4,s5,s6 -> 0..3 (8 groups/row); s7a -> 4 row0;
        # s7b -> 4 row1 (9 groups)
        stats = ctx.enter_context(
            nc.sbuf_tensor("stats", [P, 5, 2, 9, 6], mybir.dt.float32)
        )
        mv = ctx.enter_context(nc.sbuf_tensor("mv", [P, 5, 2, 2], mybir.dt.float32))
        MS = ctx.enter_context(nc.sbuf_tensor("MS", [P, 2, RPL], mybir.dt.float32))
        acc = ctx.enter_context(nc.sbuf_tensor("acc", [P, 3, 2, 3], mybir.dt.float32))
        warm = ctx.enter_context(nc.sbuf_tensor("warm", [P, 2], mybir.dt.float32))

        sslot = {0: 0, 4: 1, 5: 2, 6: 3}
        # act_done gates per epilogue, ACT emission order
        actd = {}
        cp = 0
        for s, n in [(0, 4), (1, 8), (2, 8), (3, 8), (4, 4), (5, 4), (6, 4),
                     (7, 2), (8, 2)]:
            cp += n
            actd[s] = cp
        ACT_TOTAL = cp

        def slab_src(s):  # full slabs s=0..6: 2 rows/lane
            return xl[:, 2 * s * N : (2 * s + 2) * N]

        def issue_slab(eng, s):
            eng.dma_start(out=xt[:, RING[s], :], in_=slab_src(s)).then_inc(
                dma_s[s], 16
            )

        def out_dma(eng, sem, is_std, u0, w):
            dst = bass.AP(
                tensor=out_ext,
                offset=is_std * C + u0,
                ap=[[2 * C, B_LOCAL], [RPL, P // B_LOCAL], [1, w]],
            )
            with nc.allow_non_contiguous_dma(reason="4B-per-lane stat column"):
                eng.dma_start(out=dst, in_=MS[:, is_std, u0 : u0 + w]).then_inc(
                    sem, 16
                )

        @block.sync
        def _(sync):
            issue_slab(sync, 0)
            issue_slab(sync, 2)
            sync.wait_ge(dve_stats, dve_cum[0])  # slot 0 free (s0 on DVE)
            issue_slab(sync, 5)
            sync.wait_ge(act_stats, acts_after[1])  # slot 1 free (s1 on ACT)
            issue_slab(sync, 6)
            # s7a: slot 2 (s2 on ACT), one row, 16 KiB descriptors
            sync.wait_ge(act_stats, acts_after[2])
            sync.dma_start(
                out=xt[:, RING[7], 0:N], in_=xl[:, 14 * N : 15 * N]
            ).then_inc(dma_s[7], 16)
            # s7b: slot 3 (s3 on ACT), one row, column-chunked
            sync.wait_ge(act_stats, acts_after[3])
            c0 = 0
            for g, w in enumerate(CHUNKS):
                sync.dma_start(
                    out=xt[:, RING[8], c0 : c0 + w],
                    in_=xl[:, 15 * N + c0 : 15 * N + c0 + w],
                ).then_inc(dma_c[g], 16)
                c0 += w
            # final std out (in parallel with scalar's final mean out)
            sync.wait_ge(act_done, ACT_TOTAL)
            out_dma(sync, fin_sem, 1, RPL - 1, 1)
            sync.wait_ge(out_sem, 16 * 3)
            sync.wait_ge(fin_sem, 16)

        @block.vector
        def _(vector):
            ndve = 0
            nmv = 0

            vector.memset(warm[:, :], 0.0).then_inc(warm_done, 1)

            for s in DVE_SLABS:
                ss = sslot[s]
                vector.wait_ge(dma_s[s], 16)
                for r in range(2):
                    for g in range(8):
                        vector.bn_stats(
                            out=stats[:, ss, r, g, :],
                            in_=xt[
                                :, RING[s], (r * 8 + g) * 512 : (r * 8 + g + 1) * 512
                            ],
                        ).then_inc(dve_stats, 1)
                        ndve += 1
                assert ndve == dve_cum[s]
                vector.wait_ge(dve_stats, ndve)
                for r in range(2):
                    vector.bn_aggr(
                        out=mv[:, ss, r, :], in_=stats[:, ss, r, 0:8, :]
                    ).then_inc(mv_ready, 1)
                    nmv += 1

            # s7a: one row, 8 groups
            vector.wait_ge(dma_s[7], 16)
            for g in range(8):
                vector.bn_stats(
                    out=stats[:, 4, 0, g, :],
                    in_=xt[:, RING[7], g * 512 : (g + 1) * 512],
                ).then_inc(dve_stats, 1)
                ndve += 1
            assert ndve == dve_cum[7]
            vector.wait_ge(dve_stats, ndve)
            vector.bn_aggr(out=mv[:, 4, 0, :], in_=stats[:, 4, 0, 0:8, :]).then_inc(
                mv_ready, 1
            )
            nmv += 1

            # s7b: chunk-pipelined bn_stats
            c0 = 0
            for g, w in enumerate(CHUNKS):
                vector.wait_ge(dma_c[g], 16)
                vector.bn_stats(
                    out=stats[:, 4, 1, g, :], in_=xt[:, RING[8], c0 : c0 + w]
                ).then_inc(dve_stats, 1)
                ndve += 1
                c0 += w
            vector.wait_ge(dve_stats, ndve)
            vector.bn_aggr(out=mv[:, 4, 1, :], in_=stats[:, 4, 1, :, :]).then_inc(
                mv_ready, 1
            )
            nmv += 1
            assert nmv == mv_after[8]

        @block.scalar
        def _(scalar):
            A = 1.0 / np.sqrt(float(N) * (N - 1))
            cact = 0
            nacc = 0

            # second-queue slab issues first (fresh ring slots, no waits)
            issue_slab(scalar, 1)
            issue_slab(scalar, 3)
            issue_slab(scalar, 4)

            scalar.wait_ge(warm_done, 1)
            scalar.activation(
                out=warm[:, 0:1],
                in_=warm[:, 1:2],
                func=mybir.ActivationFunctionType.Copy,
            )

            def acc_pass(s):
                nonlocal nacc
                ai = s - 1
                scalar.wait_ge(dma_s[s], 16)
                for r in range(2):
                    row = xt[:, RING[s], r * N : (r + 1) * N]
                    scalar.activation(
                        out=row,
                        in_=row,
                        func=mybir.ActivationFunctionType.Copy,
                        accum_out=acc[:, ai, r, 0:1],
                    ).then_inc(act_stats, 1)
                    nacc += 1
                    scalar.wait_ge(act_stats, nacc)
                    scalar.activation(
                        out=row,
                        in_=row,
                        func=mybir.ActivationFunctionType.Square,
                        accum_out=acc[:, ai, r, 1:2],
                    ).then_inc(act_stats, 1)
                    nacc += 1
                assert nacc == acts_after[s]

            def mv_stat_ops(mean_src, var_src, mean_dst, std_dst):
                nonlocal cact
                scalar.copy(out=mean_dst, in_=mean_src).then_inc(act_done, 1)
                scalar.activation(
                    out=std_dst,
                    in_=var_src,
                    func=mybir.ActivationFunctionType.Sqrt,
                    scale=float(N) / (N - 1),
                ).then_inc(act_done, 1)
                cact += 2

            def acc_stat_ops(ai, r, mean_dst, std_dst):
                nonlocal cact
                scalar.activation(
                    out=mean_dst,
                    in_=acc[:, ai, r, 0:1],
                    func=mybir.ActivationFunctionType.Copy,
                    scale=1.0 / N,
                ).then_inc(act_done, 1)
                scalar.activation(
                    out=acc[:, ai, r, 2:3],
                    in_=acc[:, ai, r, 0:1],
                    func=mybir.ActivationFunctionType.Square,
                    scale=A,
                ).then_inc(act_done, 1)
                cact += 2
                scalar.wait_ge(act_done, cact)
                scalar.activation(
                    out=acc[:, ai, r, 2:3],
                    in_=acc[:, ai, r, 2:3],
                    func=mybir.ActivationFunctionType.Copy,
                    scale=-1.0,
                ).then_inc(act_done, 1)
                cact += 1
                scalar.wait_ge(act_done, cact)
                scalar.activation(
                    out=std_dst,
                    in_=acc[:, ai, r, 1:2],
                    func=mybir.ActivationFunctionType.Sqrt,
                    scale=1.0 / (N - 1),
                    bias=acc[:, ai, r, 2:3],
                ).then_inc(act_done, 1)
                cact += 1

            def mdst(u):
                return MS[:, 0, u : u + 1]

            def sdst(u):
                return MS[:, 1, u : u + 1]

            def epi_full(s):
                if s in ACT_SLABS:
                    scalar.wait_ge(act_stats, acts_after[s])
                    for r in range(2):
                        acc_stat_ops(s - 1, r, mdst(2 * s + r), sdst(2 * s + r))
                else:
                    scalar.wait_ge(mv_ready, mv_after[s])
                    ss = sslot[s]
                    for r in range(2):
                        mv_stat_ops(
                            mv[:, ss, r, 0:1], mv[:, ss, r, 1:2],
                            mdst(2 * s + r), sdst(2 * s + r),
                        )
                assert cact == actd[s], (s, cact, actd[s])

            acc_pass(1)
            acc_pass(2)
            epi_full(0)
            acc_pass(3)
            epi_full(1)
            epi_full(2)
            epi_full(3)
            epi_full(4)
            epi_full(5)
            epi_full(6)
            # s7a epilogue (slot/col 14)
            scalar.wait_ge(mv_ready, mv_after[7])
            mv_stat_ops(mv[:, 4, 0, 0:1], mv[:, 4, 0, 1:2], mdst(14), sdst(14))
            assert cact == actd[7]
            # early out: mean+std for slots 0..14 (hidden mid-stream)
            scalar.wait_ge(act_done, actd[7])
            out_dma(scalar, out_sem, 0, 0, RPL - 1)
            out_dma(scalar, out_sem, 1, 0, RPL - 1)
            # s7b epilogue (slot/col 15) + final mean out
            scalar.wait_ge(mv_ready, mv_after[8])
            mv_stat_ops(mv[:, 4, 1, 0:1], mv[:, 4, 1, 1:2], mdst(15), sdst(15))
            assert cact == actd[8] == ACT_TOTAL
            scalar.wait_ge(act_done, ACT_TOTAL)
            out_dma(scalar, out_sem, 0, RPL - 1, 1)

    return nc


def kernel(f_vol: np.ndarray) -> np.ndarray:
    from concourse.bass_utils import run_bass_kernel_spmd

    if "nc" not in _CACHE:
        _CACHE["nc"] = _build()
    nc = _CACHE["nc"]

    f_vol = np.ascontiguousarray(f_vol, dtype=np.float32)
    in_maps = [
        {"f_vol": f_vol[i * B_LOCAL : (i + 1) * B_LOCAL]} for i in range(N_CORES)
    ]
    res = run_bass_kernel_spmd(nc, in_maps, core_ids=list(range(N_CORES)))
    return np.concatenate([res.results[i]["out"] for i in range(N_CORES)], axis=0)


# revision 18
# speedup vs baseline: 1.4558x; 1.0762x over previous
"""AdaIN statistics kernel for TRN2, SPMD across 8 NeuronCores. v7.

Input : f_vol [32, 512, 64, 64] f32
Output: [32, 1024] f32 = concat([mean over (h,w), unbiased std over (h,w)], axis=-1)

Sharding: data-parallel over batch - each of the 8 cores handles 4 batches
([4, 512, 64, 64] shard, 32 MiB). No collectives; the host concatenates the
8 per-core [4, 1024] outputs.

DMA facts established by v2-v6 traces on this part:
  - Only uniform 128-partition DMAs with >=8 KiB per-partition descriptors
    run at per-engine line rate (~26.7 B/ns).  Sub-128-partition DMAs are
    assigned positionally (desc i -> engine i for n<=16) and collapse the
    whole queue to ~14 B/ns (v5).  2 KiB descriptors run at ~13-21 B/ns.
  - SDMA engine column 15 runs ~21% slower than the others whenever all
    16 are loaded, on BOTH HWDGE queues (v6), so it sets the stream end:
    its 1/16 share (2.1 MiB) takes ~97 us against ~79 us for the rest.
    No descriptor layout can shift bytes off it without the small-DMA
    penalty, so the kernel accepts its pace and optimizes everything else.
  - Whole-slab completion semaphores only fire when engine 15 drains that
    slab, so whole-slab-gated compute bursts AFTER the stream (v6's 23 us
    tail).  v7 therefore streams all DVE-consumed rows as 2048-column
    chunks (8 KiB descriptors - still line rate) with per-chunk
    semaphores: DVE tracks engine 15's pace and finishes ~1 us after the
    last byte.

Lane-major deal: lane p owns rows 16p..16p+15 (row = b*512+c of the
[2048, 4096] row-major view); slot u = lane-local row u = output column
16*(p%32)+u of batch p//32.  ACT consumes slots 0-5 as three whole
2-row slabs (Copy/Square+accumulate, 32 KiB descriptors, placed early in
the queue); DVE consumes slots 6-15 as chunked single rows.  The final
row's chunks shrink (2048/1024/512/384/128) so the exposed tail is one
128-col bn_stats + bn_aggr + 2 ACT ops + 2 tiny out-DMAs (mean from
scalar, std from sync, in parallel).

Outputs accumulate in MS[128, 2, 16]; the whole output is 4 DMAs:
early mean/std for slots 0..14 (hidden mid-stream), final mean/std for
slot 15.

Semaphore discipline (v2 lesson): same-engine RAW through SBUF is NOT
covered by program order; every producer->consumer edge carries an
explicit semaphore observation.  Semaphores and stats buffers are never
reused.
"""

from contextlib import ExitStack

import numpy as np

B, C, H, W = 32, 512, 64, 64
N_CORES = 8
B_LOCAL = B // N_CORES  # 4
N = H * W  # 4096
P = 128
ROWS = B_LOCAL * C  # 2048
RPL = ROWS // P  # 16 rows (slots) per lane

NACT = 3  # ACT slab pairs: slots (0,1), (2,3), (4,5)
DROWS = list(range(6, 16))  # DVE rows (slots)
NR = 4  # DVE row ring slots
ROW_CHUNKS = {u: [2048, 2048] for u in DROWS}
ROW_CHUNKS[15] = [2048, 1024, 512, 384, 128]
GROUPW = {u: [512] * 8 for u in DROWS}
GROUPW[15] = [512] * 7 + [384, 128]
for u in DROWS:
    assert sum(ROW_CHUNKS[u]) == N and sum(GROUPW[u]) == N
# queue order: ACT slabs early but interleaved so DVE rows start arriving soon
QORDER = ["a0", 6, "a1", 7, 8, "a2", 9, 10, 11, 12, 13, 14, 15]

_CACHE = {}


def _build():
    import concourse.bass as bass
    from concourse import mybir

    nc = bass.Bass()
    x_ext = nc.declare_dram_parameter(
        "f_vol", [B_LOCAL, C, H, W], mybir.dt.float32, isOutput=False
    )
    out_ext = nc.declare_dram_parameter(
        "out", [B_LOCAL, 2 * C], mybir.dt.float32, isOutput=True
    )

    xl = (
        x_ext.ap()
        .rearrange("b c h w -> (b c) (h w)")
        .rearrange("(p u) f -> p (u f)", u=RPL)
    )

    # DVE cumulative bn_stats counts after each row
    D = {}
    c = 0
    for u in DROWS:
        c += len(GROUPW[u])
        D[u] = c
    DVE_TOTAL = c  # 81
    MV = {u: u - 5 for u in DROWS}  # mv_ready after row u
    ACTS = {a: 4 * (a + 1) for a in range(NACT)}  # act_stats after acc pair a

    with ExitStack() as ctx:
        block = ctx.enter_context(nc.Block(no_gpsimd_drain=True))
        dma_a = [ctx.enter_context(nc.semaphore(f"dma_a{a}")) for a in range(NACT)]
        dma_rc = {
            u: [
                ctx.enter_context(nc.semaphore(f"dma_r{u}c{i}"))
                for i in range(len(ROW_CHUNKS[u]))
            ]
            for u in DROWS
        }
        out_sem = ctx.enter_context(nc.semaphore("out_sem"))
        fin_sem = ctx.enter_context(nc.semaphore("fin_sem"))
        dve_stats = ctx.enter_context(nc.semaphore("dve_stats"))
        mv_ready = ctx.enter_context(nc.semaphore("mv_ready"))
        act_stats = ctx.enter_context(nc.semaphore("act_stats"))
        act_done = ctx.enter_context(nc.semaphore("act_done"))
        warm_done = ctx.enter_context(nc.semaphore("warm_done"))

        xtA = ctx.enter_context(
            nc.sbuf_tensor("xtA", [P, NACT, 2 * N], mybir.dt.float32)
        )
        xtR = ctx.enter_context(nc.sbuf_tensor("xtR", [P, NR, N], mybir.dt.float32))
        stats = ctx.enter_context(
            nc.sbuf_tensor("stats", [P, len(DROWS), 9, 6], mybir.dt.float32)
        )
        mv = ctx.enter_context(
            nc.sbuf_tensor("mv", [P, len(DROWS), 2], mybir.dt.float32)
        )
        MS = ctx.enter_context(nc.sbuf_tensor("MS", [P, 2, RPL], mybir.dt.float32))
        acc = ctx.enter_context(
            nc.sbuf_tensor("acc", [P, NACT, 2, 3], mybir.dt.float32)
        )
        warm = ctx.enter_context(nc.sbuf_tensor("warm", [P, 2], mybir.dt.float32))

        # act_done gates: epi_acc(a)=8 ops; epi_mv(u)=2 ops; emission order:
        # epi_acc 0,1,2 then epi_mv 6..14, early out, epi_mv 15, final out
        actd_acc = {a: 8 * (a + 1) for a in range(NACT)}
        actd_mv = {u: 24 + 2 * (u - 5) for u in DROWS}
        ACT_TOTAL = actd_mv[15]  # 44

        def rslot(u):
            return (u - 6) % NR

        def out_dma(eng, sem, is_std, u0, w):
            dst = bass.AP(
                tensor=out_ext,
                offset=is_std * C + u0,
                ap=[[2 * C, B_LOCAL], [RPL, P // B_LOCAL], [1, w]],
            )
            with nc.allow_non_contiguous_dma(reason="4B-per-lane stat column"):
                eng.dma_start(out=dst, in_=MS[:, is_std, u0 : u0 + w]).then_inc(
                    sem, 16
                )

        @block.sync
        def _(sync):
            for item in QORDER:
                if isinstance(item, str):
                    a = int(item[1])
                    sync.dma_start(
                        out=xtA[:, a, :], in_=xl[:, 2 * a * N : (2 * a + 2) * N]
                    ).then_inc(dma_a[a], 16)
                else:
                    u = item
                    if u - 6 >= NR:  # ring reuse: row u-NR consumed by DVE
                        sync.wait_ge(dve_stats, D[u - NR])
                    c0 = 0
                    for i, w in enumerate(ROW_CHUNKS[u]):
                        sync.dma_start(
                            out=xtR[:, rslot(u), c0 : c0 + w],
                            in_=xl[:, u * N + c0 : u * N + c0 + w],
                        ).then_inc(dma_rc[u][i], 16)
                        c0 += w
            # final std out for slot 15, in parallel with scalar's mean out
            sync.wait_ge(act_done, ACT_TOTAL)
            out_dma(sync, fin_sem, 1, RPL - 1, 1)
            sync.wait_ge(out_sem, 16 * 3)
            sync.wait_ge(fin_sem, 16)

        @block.vector
        def _(vector):
            ndve = 0
            nmv = 0

            vector.memset(warm[:, :], 0.0).then_inc(warm_done, 1)

            for u in DROWS:
                si = u - 6
                gi = 0
                c0 = 0
                for i, w in enumerate(ROW_CHUNKS[u]):
                    vector.wait_ge(dma_rc[u][i], 16)
                    gg = c0
                    while gg < c0 + w:
                        gw = GROUPW[u][gi]
                        vector.bn_stats(
                            out=stats[:, si, gi, :],
                            in_=xtR[:, rslot(u), gg : gg + gw],
                        ).then_inc(dve_stats, 1)
                        ndve += 1
                        gg += gw
                        gi += 1
                    assert gg == c0 + w
                    c0 += w
                assert gi == len(GROUPW[u]) and ndve == D[u]
                vector.wait_ge(dve_stats, ndve)
                vector.bn_aggr(
                    out=mv[:, si, :], in_=stats[:, si, 0 : len(GROUPW[u]), :]
                ).then_inc(mv_ready, 1)
                nmv += 1
                assert nmv == MV[u]

        @block.scalar
        def _(scalar):
            A = 1.0 / np.sqrt(float(N) * (N - 1))
            cact = 0
            nacc = 0

            scalar.wait_ge(warm_done, 1)
            scalar.activation(
                out=warm[:, 0:1],
                in_=warm[:, 1:2],
                func=mybir.ActivationFunctionType.Copy,
            )

            def acc_pass(a):
                nonlocal nacc
                scalar.wait_ge(dma_a[a], 16)
                for r in range(2):
                    row = xtA[:, a, r * N : (r + 1) * N]
                    scalar.activation(
                        out=row,
                        in_=row,
                        func=mybir.ActivationFunctionType.Copy,
                        accum_out=acc[:, a, r, 0:1],
                    ).then_inc(act_stats, 1)
                    nacc += 1
                    scalar.wait_ge(act_stats, nacc)
                    scalar.activation(
                        out=row,
                        in_=row,
                        func=mybir.ActivationFunctionType.Square,
                        accum_out=acc[:, a, r, 1:2],
                    ).then_inc(act_stats, 1)
                    nacc += 1
                assert nacc == ACTS[a]

            def mdst(u):
                return MS[:, 0, u : u + 1]

            def sdst(u):
                return MS[:, 1, u : u + 1]

            def epi_acc(a):
                nonlocal cact
                scalar.wait_ge(act_stats, ACTS[a])
                for r in range(2):
                    u = 2 * a + r
                    scalar.activation(
                        out=mdst(u),
                        in_=acc[:, a, r, 0:1],
                        func=mybir.ActivationFunctionType.Copy,
                        scale=1.0 / N,
                    ).then_inc(act_done, 1)
                    scalar.activation(
                        out=acc[:, a, r, 2:3],
                        in_=acc[:, a, r, 0:1],
                        func=mybir.ActivationFunctionType.Square,
                        scale=A,
                    ).then_inc(act_done, 1)
                    cact += 2
                    scalar.wait_ge(act_done, cact)
                    scalar.activation(
                        out=acc[:, a, r, 2:3],
                        in_=acc[:, a, r, 2:3],
                        func=mybir.ActivationFunctionType.Copy,
                        scale=-1.0,
                    ).then_inc(act_done, 1)
                    cact += 1
                    scalar.wait_ge(act_done, cact)
                    scalar.activation(
                        out=sdst(u),
                        in_=acc[:, a, r, 1:2],
                        func=mybir.ActivationFunctionType.Sqrt,
                        scale=1.0 / (N - 1),
                        bias=acc[:, a, r, 2:3],
                    ).then_inc(act_done, 1)
                    cact += 1
                assert cact == actd_acc[a]

            def epi_mv(u):
                nonlocal cact
                scalar.wait_ge(mv_ready, MV[u])
                si = u - 6
                scalar.copy(out=mdst(u), in_=mv[:, si, 0:1]).then_inc(act_done, 1)
                scalar.activation(
                    out=sdst(u),
                    in_=mv[:, si, 1:2],
                    func=mybir.ActivationFunctionType.Sqrt,
                    scale=float(N) / (N - 1),
                ).then_inc(act_done, 1)
                cact += 2
                assert cact == actd_mv[u]

            acc_pass(0)
            acc_pass(1)
            epi_acc(0)
            acc_pass(2)
            epi_acc(1)
            epi_acc(2)
            for u in range(6, 15):
                epi_mv(u)
            # early out: mean+std for slots 0..14 (hidden mid-stream)
            scalar.wait_ge(act_done, actd_mv[14])
            out_dma(scalar, out_sem, 0, 0, RPL - 1)
            out_dma(scalar, out_sem, 1, 0, RPL - 1)
            epi_mv(15)
            scalar.wait_ge(act_done, ACT_TOTAL)
            out_dma(scalar, out_sem, 0, RPL - 1, 1)

    return nc


def kernel(f_vol: np.ndarray) -> np.ndarray:
    from concourse.bass_utils import run_bass_kernel_spmd

    if "nc" not in _CACHE:
        _CACHE["nc"] = _build()
    nc = _CACHE["nc"]

    f_vol = np.ascontiguousarray(f_vol, dtype=np.float32)
    in_maps = [
        {"f_vol": f_vol[i * B_LOCAL : (i + 1) * B_LOCAL]} for i in range(N_CORES)
    ]
    res = run_bass_kernel_spmd(nc, in_maps, core_ids=list(range(N_CORES)))
    return np.concatenate([res.results[i]["out"] for i in range(N_CORES)], axis=0)


# revision 19
# speedup vs baseline: 1.6751x; 1.1507x over previous
"""AdaIN statistics kernel for TRN2, SPMD across 8 NeuronCores. v8.

Input : f_vol [32, 512, 64, 64] f32
Output: [32, 1024] f32 = concat([mean over (h,w), unbiased std over (h,w)], axis=-1)

Sharding: data-parallel over batch - each of the 8 cores handles 4 batches
([4, 512, 64, 64] shard, 32 MiB). No collectives; the host concatenates the
8 per-core [4, 1024] outputs.

DMA facts established by v2-v7 traces on this part:
  - Per-engine stream rate depends on descriptor size: ~26.5 B/ns at
    32 KiB/partition, ~20.6 at 8 KiB, ~14 with sub-128-partition DMAs.
    So the bulk must ride 32 KiB-descriptor full-128-lane slabs.
  - SDMA engine column 15 runs ~21 B/ns whenever all 16 are loaded (both
    HWDGE queues), so its 1/16 byte share sets the stream end (~106 us).
    No descriptor layout shifts bytes off it without the small-DMA
    penalty (v5), so the kernel accepts its pace.
  - Whole-slab completion semaphores fire only when engine 15 drains the
    slab, so slab-gated compute lags the stream.  Pair-slabs complete at
    ~11.9 us spacing vs ~10.9 us of DVE work per pair - DVE keeps pace
    at slab granularity; only the last two rows need finer chunking so
    the exposed tail is one 128-col bn_stats + bn_aggr + 2 ACT ops + 2
    tiny out-DMAs (mean from scalar, std from sync, in parallel).

Lane-major deal: lane p owns rows 16p..16p+15 (row = b*512+c of the
[2048, 4096] row-major view); slot u = output column 16*(p%32)+u of
batch p//32.  ACT consumes slots 0-5 as three whole pair-slabs
(Copy/Square+accumulate); DVE consumes slots 6-13 as four pair-slabs,
slot 14 as one single-row DMA (16 KiB descriptors, arrives ~8 us before
stream end, stats hidden), slot 15 chunked 2048/1024/512/384/128.

Outputs accumulate in MS[128, 2, 16]; the whole output is 4 DMAs: early
mean/std for slots 0..14 (hidden mid-stream), final mean/std for slot 15.

Semaphore discipline (v2 lesson): same-engine RAW through SBUF is NOT
covered by program order; every producer->consumer edge carries an
explicit semaphore observation.  Semaphores and stats buffers are never
reused.  SBUF reuse: DVE pair d3 lands in ACT slab 0's buffer and rows
14/15 in ACT slab 1's, gated on act_stats (ACT's reads retired).
"""

from contextlib import ExitStack

import numpy as np

B, C, H, W = 32, 512, 64, 64
N_CORES = 8
B_LOCAL = B // N_CORES  # 4
N = H * W  # 4096
P = 128
ROWS = B_LOCAL * C  # 2048
RPL = ROWS // P  # 16 rows (slots) per lane

NACT = 3  # ACT pair-slabs: slots (0,1), (2,3), (4,5)
NDVE = 4  # DVE pair-slabs: slots (6,7), (8,9), (10,11), (12,13)
R15_CHUNKS = [2048, 1024, 512, 384, 128]
GROUP15 = [512] * 7 + [384, 128]
assert sum(R15_CHUNKS) == N and sum(GROUP15) == N

_CACHE = {}


def _build():
    import concourse.bass as bass
    from concourse import mybir

    nc = bass.Bass()
    x_ext = nc.declare_dram_parameter(
        "f_vol", [B_LOCAL, C, H, W], mybir.dt.float32, isOutput=False
    )
    out_ext = nc.declare_dram_parameter(
        "out", [B_LOCAL, 2 * C], mybir.dt.float32, isOutput=True
    )

    xl = (
        x_ext.ap()
        .rearrange("b c h w -> (b c) (h w)")
        .rearrange("(p u) f -> p (u f)", u=RPL)
    )

    # DVE cumulative bn_stats after each unit: pairs 16 each, r14 8, r15 9
    D_PAIR = {k: 16 * (k + 1) for k in range(NDVE)}  # 16..64
    D_R14 = 72
    DVE_TOTAL = 81
    # mv_ready after row u (slots 6..15 -> 1..10)
    MV = {u: u - 5 for u in range(6, 16)}
    ACTS = {a: 4 * (a + 1) for a in range(NACT)}

    with ExitStack() as ctx:
        block = ctx.enter_context(nc.Block(no_gpsimd_drain=True))
        dma_a = [ctx.enter_context(nc.semaphore(f"dma_a{a}")) for a in range(NACT)]
        dma_d = [ctx.enter_context(nc.semaphore(f"dma_d{k}")) for k in range(NDVE)]
        dma_r14 = ctx.enter_context(nc.semaphore("dma_r14"))
        dma_rc = [
            ctx.enter_context(nc.semaphore(f"dma_r15c{i}"))
            for i in range(len(R15_CHUNKS))
        ]
        out_sem = ctx.enter_context(nc.semaphore("out_sem"))
        fin_sem = ctx.enter_context(nc.semaphore("fin_sem"))
        dve_stats = ctx.enter_context(nc.semaphore("dve_stats"))
        mv_ready = ctx.enter_context(nc.semaphore("mv_ready"))
        act_stats = ctx.enter_context(nc.semaphore("act_stats"))
        act_done = ctx.enter_context(nc.semaphore("act_done"))
        warm_done = ctx.enter_context(nc.semaphore("warm_done"))

        xtA = ctx.enter_context(
            nc.sbuf_tensor("xtA", [P, NACT, 2 * N], mybir.dt.float32)
        )
        xtD = ctx.enter_context(nc.sbuf_tensor("xtD", [P, 3, 2 * N], mybir.dt.float32))
        stats = ctx.enter_context(
            nc.sbuf_tensor("stats", [P, 10, 9, 6], mybir.dt.float32)
        )
        mv = ctx.enter_context(nc.sbuf_tensor("mv", [P, 10, 2], mybir.dt.float32))
        MS = ctx.enter_context(nc.sbuf_tensor("MS", [P, 2, RPL], mybir.dt.float32))
        acc = ctx.enter_context(
            nc.sbuf_tensor("acc", [P, NACT, 2, 3], mybir.dt.float32)
        )
        warm = ctx.enter_context(nc.sbuf_tensor("warm", [P, 2], mybir.dt.float32))

        # DVE pair k buffer: k<3 -> xtD slot k; k=3 -> xtA slot 0 (reused).
        # Row 14 -> xtA slot 1 low half; row 15 -> xtA slot 1 high half.
        def dbuf(k):
            return xtD[:, k, :] if k < 3 else xtA[:, 0, :]

        r14_buf = xtA[:, 1, 0:N]
        r15_buf = xtA[:, 1, N : 2 * N]

        # act_done gates: epi_acc(a)=8; epi_mv(u)=2; order: acc epis 0..2,
        # mv epis 6..14, early out, mv epi 15, final out
        actd_acc = {a: 8 * (a + 1) for a in range(NACT)}
        actd_mv = {u: 24 + 2 * (u - 5) for u in range(6, 16)}
        ACT_TOTAL = actd_mv[15]  # 44

        def out_dma(eng, sem, is_std, u0, w):
            dst = bass.AP(
                tensor=out_ext,
                offset=is_std * C + u0,
                ap=[[2 * C, B_LOCAL], [RPL, P // B_LOCAL], [1, w]],
            )
            with nc.allow_non_contiguous_dma(reason="4B-per-lane stat column"):
                eng.dma_start(out=dst, in_=MS[:, is_std, u0 : u0 + w]).then_inc(
                    sem, 16
                )

        @block.sync
        def _(sync):
            def pair_src(s0):  # slots (s0, s0+1)
                return xl[:, s0 * N : (s0 + 2) * N]

            # interleave ACT and DVE pair-slabs
            sync.dma_start(out=xtA[:, 0, :], in_=pair_src(0)).then_inc(dma_a[0], 16)
            sync.dma_start(out=dbuf(0), in_=pair_src(6)).then_inc(dma_d[0], 16)
            sync.dma_start(out=xtA[:, 1, :], in_=pair_src(2)).then_inc(dma_a[1], 16)
            sync.dma_start(out=dbuf(1), in_=pair_src(8)).then_inc(dma_d[1], 16)
            sync.dma_start(out=xtA[:, 2, :], in_=pair_src(4)).then_inc(dma_a[2], 16)
            sync.dma_start(out=dbuf(2), in_=pair_src(10)).then_inc(dma_d[2], 16)
            # d3 reuses ACT slab 0's buffer; ACT finished reading it at
            # act_stats >= 4
            sync.wait_ge(act_stats, ACTS[0])
            sync.dma_start(out=dbuf(3), in_=pair_src(12)).then_inc(dma_d[3], 16)
            # rows 14/15 reuse ACT slab 1's buffer
            sync.wait_ge(act_stats, ACTS[1])
            sync.dma_start(out=r14_buf, in_=xl[:, 14 * N : 15 * N]).then_inc(
                dma_r14, 16
            )
            c0 = 0
            for i, w in enumerate(R15_CHUNKS):
                sync.dma_start(
                    out=r15_buf[:, c0 : c0 + w],
                    in_=xl[:, 15 * N + c0 : 15 * N + c0 + w],
                ).then_inc(dma_rc[i], 16)
                c0 += w
            # final std out for slot 15, parallel with scalar's mean out
            sync.wait_ge(act_done, ACT_TOTAL)
            out_dma(sync, fin_sem, 1, RPL - 1, 1)
            sync.wait_ge(out_sem, 16 * 3)
            sync.wait_ge(fin_sem, 16)

        @block.vector
        def _(vector):
            ndve = 0
            nmv = 0

            vector.memset(warm[:, :], 0.0).then_inc(warm_done, 1)

            for k in range(NDVE):
                vector.wait_ge(dma_d[k], 16)
                buf = dbuf(k)
                for r in range(2):
                    si = 2 * k + r  # stats index for slot u = 6+2k+r
                    for g in range(8):
                        vector.bn_stats(
                            out=stats[:, si, g, :],
                            in_=buf[:, (r * 8 + g) * 512 : (r * 8 + g + 1) * 512],
                        ).then_inc(dve_stats, 1)
                        ndve += 1
                assert ndve == D_PAIR[k]
                vector.wait_ge(dve_stats, ndve)
                for r in range(2):
                    si = 2 * k + r
                    vector.bn_aggr(
                        out=mv[:, si, :], in_=stats[:, si, 0:8, :]
                    ).then_inc(mv_ready, 1)
                    nmv += 1
                assert nmv == MV[6 + 2 * k + 1]

            # row 14: one DMA, 8 groups
            vector.wait_ge(dma_r14, 16)
            for g in range(8):
                vector.bn_stats(
                    out=stats[:, 8, g, :], in_=r14_buf[:, g * 512 : (g + 1) * 512]
                ).then_inc(dve_stats, 1)
                ndve += 1
            assert ndve == D_R14
            vector.wait_ge(dve_stats, ndve)
            vector.bn_aggr(out=mv[:, 8, :], in_=stats[:, 8, 0:8, :]).then_inc(
                mv_ready, 1
            )
            nmv += 1

            # row 15: chunk-paced
            gi = 0
            c0 = 0
            for i, w in enumerate(R15_CHUNKS):
                vector.wait_ge(dma_rc[i], 16)
                gg = c0
                while gg < c0 + w:
                    gw = GROUP15[gi]
                    vector.bn_stats(
                        out=stats[:, 9, gi, :], in_=r15_buf[:, gg : gg + gw]
                    ).then_inc(dve_stats, 1)
                    ndve += 1
                    gg += gw
                    gi += 1
                c0 += w
            assert gi == len(GROUP15) and ndve == DVE_TOTAL
            vector.wait_ge(dve_stats, ndve)
            vector.bn_aggr(
                out=mv[:, 9, :], in_=stats[:, 9, 0 : len(GROUP15), :]
            ).then_inc(mv_ready, 1)
            nmv += 1
            assert nmv == MV[15]

        @block.scalar
        def _(scalar):
            A = 1.0 / np.sqrt(float(N) * (N - 1))
            cact = 0
            nacc = 0

            scalar.wait_ge(warm_done, 1)
            scalar.activation(
                out=warm[:, 0:1],
                in_=warm[:, 1:2],
                func=mybir.ActivationFunctionType.Copy,
            )

            def acc_pass(a):
                nonlocal nacc
                scalar.wait_ge(dma_a[a], 16)
                for r in range(2):
                    row = xtA[:, a, r * N : (r + 1) * N]
                    scalar.activation(
                        out=row,
                        in_=row,
                        func=mybir.ActivationFunctionType.Copy,
                        accum_out=acc[:, a, r, 0:1],
                    ).then_inc(act_stats, 1)
                    nacc += 1
                    scalar.wait_ge(act_stats, nacc)
                    scalar.activation(
                        out=row,
                        in_=row,
                        func=mybir.ActivationFunctionType.Square,
                        accum_out=acc[:, a, r, 1:2],
                    ).then_inc(act_stats, 1)
                    nacc += 1
                assert nacc == ACTS[a]

            def mdst(u):
                return MS[:, 0, u : u + 1]

            def sdst(u):
                return MS[:, 1, u : u + 1]

            def epi_acc(a):
                nonlocal cact
                scalar.wait_ge(act_stats, ACTS[a])
                for r in range(2):
                    u = 2 * a + r
                    scalar.activation(
                        out=mdst(u),
                        in_=acc[:, a, r, 0:1],
                        func=mybir.ActivationFunctionType.Copy,
                        scale=1.0 / N,
                    ).then_inc(act_done, 1)
                    scalar.activation(
                        out=acc[:, a, r, 2:3],
                        in_=acc[:, a, r, 0:1],
                        func=mybir.ActivationFunctionType.Square,
                        scale=A,
                    ).then_inc(act_done, 1)
                    cact += 2
                    scalar.wait_ge(act_done, cact)
                    scalar.activation(
                        out=acc[:, a, r, 2:3],
                        in_=acc[:, a, r, 2:3],
                        func=mybir.ActivationFunctionType.Copy,
                        scale=-1.0,
                    ).then_inc(act_done, 1)
                    cact += 1
                    scalar.wait_ge(act_done, cact)
                    scalar.activation(
                        out=sdst(u),
                        in_=acc[:, a, r, 1:2],
                        func=mybir.ActivationFunctionType.Sqrt,
                        scale=1.0 / (N - 1),
                        bias=acc[:, a, r, 2:3],
                    ).then_inc(act_done, 1)
                    cact += 1
                assert cact == actd_acc[a]

            def epi_mv(u):
                nonlocal cact
                scalar.wait_ge(mv_ready, MV[u])
                si = u - 6
                scalar.copy(out=mdst(u), in_=mv[:, si, 0:1]).then_inc(act_done, 1)
                scalar.activation(
                    out=sdst(u),
                    in_=mv[:, si, 1:2],
                    func=mybir.ActivationFunctionType.Sqrt,
                    scale=float(N) / (N - 1),
                ).then_inc(act_done, 1)
                cact += 2
                assert cact == actd_mv[u]

            acc_pass(0)
            acc_pass(1)
            epi_acc(0)
            acc_pass(2)
            epi_acc(1)
            epi_acc(2)
            for u in range(6, 15):
                epi_mv(u)
            # early out: mean+std for slots 0..14 (hidden mid-stream)
            scalar.wait_ge(act_done, actd_mv[14])
            out_dma(scalar, out_sem, 0, 0, RPL - 1)
            out_dma(scalar, out_sem, 1, 0, RPL - 1)
            epi_mv(15)
            scalar.wait_ge(act_done, ACT_TOTAL)
            out_dma(scalar, out_sem, 0, RPL - 1, 1)

    return nc


def kernel(f_vol: np.ndarray) -> np.ndarray:
    from concourse.bass_utils import run_bass_kernel_spmd

    if "nc" not in _CACHE:
        _CACHE["nc"] = _build()
    nc = _CACHE["nc"]

    f_vol = np.ascontiguousarray(f_vol, dtype=np.float32)
    in_maps = [
        {"f_vol": f_vol[i * B_LOCAL : (i + 1) * B_LOCAL]} for i in range(N_CORES)
    ]
    res = run_bass_kernel_spmd(nc, in_maps, core_ids=list(range(N_CORES)))
    return np.concatenate([res.results[i]["out"] for i in range(N_CORES)], axis=0)


# revision 25
# speedup vs baseline: 1.7602x; 1.0508x over previous
"""AdaIN statistics kernel for TRN2, SPMD across 8 NeuronCores. v8.

Input : f_vol [32, 512, 64, 64] f32
Output: [32, 1024] f32 = concat([mean over (h,w), unbiased std over (h,w)], axis=-1)

Sharding: data-parallel over batch - each of the 8 cores handles 4 batches
([4, 512, 64, 64] shard, 32 MiB). No collectives; the host concatenates the
8 per-core [4, 1024] outputs.

DMA facts established by v2-v7 traces on this part:
  - Per-engine stream rate depends on descriptor size: ~26.5 B/ns at
    32 KiB/partition, ~20.6 at 8 KiB, ~14 with sub-128-partition DMAs.
    So the bulk must ride 32 KiB-descriptor full-128-lane slabs.
  - SDMA engine column 15 runs ~21 B/ns whenever all 16 are loaded (both
    HWDGE queues), so its 1/16 byte share sets the stream end (~106 us).
    No descriptor layout shifts bytes off it without the small-DMA
    penalty (v5), so the kernel accepts its pace.
  - Whole-slab completion semaphores fire only when engine 15 drains the
    slab, so slab-gated compute lags the stream.  Pair-slabs complete at
    ~11.9 us spacing vs ~10.9 us of DVE work per pair - DVE keeps pace
    at slab granularity; only the last two rows need finer chunking so
    the exposed tail is one 128-col bn_stats + bn_aggr + 2 ACT ops + 2
    tiny out-DMAs (mean from scalar, std from sync, in parallel).

Lane-major deal: lane p owns rows 16p..16p+15 (row = b*512+c of the
[2048, 4096] row-major view); slot u = output column 16*(p%32)+u of
batch p//32.  ACT consumes slots 0-5 as three whole pair-slabs
(Copy/Square+accumulate); DVE consumes slots 6-13 as four pair-slabs,
slot 14 as one single-row DMA (16 KiB descriptors, arrives ~8 us before
stream end, stats hidden), slot 15 chunked 2048/1024/512/384/128.

Outputs accumulate in MS[128, 2, 16]; the whole output is 4 DMAs: early
mean/std for slots 0..14 (hidden mid-stream), final mean/std for slot 15.

Semaphore discipline (v2 lesson): same-engine RAW through SBUF is NOT
covered by program order; every producer->consumer edge carries an
explicit semaphore observation.  Semaphores and stats buffers are never
reused.  SBUF reuse: DVE pair d3 lands in ACT slab 0's buffer and rows
14/15 in ACT slab 1's, gated on act_stats (ACT's reads retired).
"""

from contextlib import ExitStack

import numpy as np

B, C, H, W = 32, 512, 64, 64
N_CORES = 8
B_LOCAL = B // N_CORES  # 4
N = H * W  # 4096
P = 128
ROWS = B_LOCAL * C  # 2048
RPL = ROWS // P  # 16 rows (slots) per lane

NACT = 3  # ACT pair-slabs: slots (0,1), (2,3), (4,5)
NDVE = 4  # DVE pair-slabs: slots (6,7), (8,9), (10,11), (12,13)
R15_CHUNKS = [2048, 1024, 512, 384, 128]
GROUP15 = [512] * 7 + [384, 128]
assert sum(R15_CHUNKS) == N and sum(GROUP15) == N

_CACHE = {}


def _build():
    import concourse.bass as bass
    from concourse import mybir

    nc = bass.Bass()
    x_ext = nc.declare_dram_parameter(
        "f_vol", [B_LOCAL, C, H, W], mybir.dt.float32, isOutput=False
    )
    out_ext = nc.declare_dram_parameter(
        "out", [B_LOCAL, 2 * C], mybir.dt.float32, isOutput=True
    )

    xl = (
        x_ext.ap()
        .rearrange("b c h w -> (b c) (h w)")
        .rearrange("(p u) f -> p (u f)", u=RPL)
    )

    # DVE cumulative bn_stats after each unit: pairs d0-d2 16 each; d3 is
    # split (DVE row 12 only, 8 stats - ACT consumes row 13 so DVE's
    # terminal backlog shrinks by ~5 us); r14 8; r15 9.
    D_PAIR = {0: 16, 1: 32, 2: 48, 3: 56}
    D_R14 = 64
    DVE_TOTAL = 73
    # mv_ready: rows 6..12 -> 1..7 (row 13 is acc-form), r14 -> 8, r15 -> 9
    MV = {u: u - 5 for u in range(6, 13)}
    MV[14] = 8
    MV[15] = 9
    # act_stats: acc pairs 0-2 -> 4, 8, 12; row-13 pass -> 14
    ACTS = {a: 4 * (a + 1) for a in range(NACT)}
    ACTS_R13 = 14

    with ExitStack() as ctx:
        block = ctx.enter_context(nc.Block(no_gpsimd_drain=True))
        dma_a = [ctx.enter_context(nc.semaphore(f"dma_a{a}")) for a in range(NACT)]
        dma_d = [ctx.enter_context(nc.semaphore(f"dma_d{k}")) for k in range(NDVE)]
        dma_r14 = ctx.enter_context(nc.semaphore("dma_r14"))
        dma_rc = [
            ctx.enter_context(nc.semaphore(f"dma_r15c{i}"))
            for i in range(len(R15_CHUNKS))
        ]
        out_sem = ctx.enter_context(nc.semaphore("out_sem"))
        fin_sem = ctx.enter_context(nc.semaphore("fin_sem"))
        dve_stats = ctx.enter_context(nc.semaphore("dve_stats"))
        mv_ready = ctx.enter_context(nc.semaphore("mv_ready"))
        act_stats = ctx.enter_context(nc.semaphore("act_stats"))
        act_done = ctx.enter_context(nc.semaphore("act_done"))
        warm_done = ctx.enter_context(nc.semaphore("warm_done"))

        xtA = ctx.enter_context(
            nc.sbuf_tensor("xtA", [P, NACT, 2 * N], mybir.dt.float32)
        )
        xtD = ctx.enter_context(nc.sbuf_tensor("xtD", [P, 3, 2 * N], mybir.dt.float32))
        stats = ctx.enter_context(
            nc.sbuf_tensor("stats", [P, 10, 9, 6], mybir.dt.float32)
        )
        mv = ctx.enter_context(nc.sbuf_tensor("mv", [P, 10, 2], mybir.dt.float32))
        MS = ctx.enter_context(nc.sbuf_tensor("MS", [P, 2, RPL], mybir.dt.float32))
        acc = ctx.enter_context(
            nc.sbuf_tensor("acc", [P, NACT + 1, 2, 3], mybir.dt.float32)
        )
        warm = ctx.enter_context(nc.sbuf_tensor("warm", [P, 2], mybir.dt.float32))

        # DVE pair k buffer: k<3 -> xtD slot k; k=3 -> xtA slot 0 (reused).
        # Row 14 -> xtA slot 1 low half; row 15 -> xtA slot 1 high half.
        def dbuf(k):
            return xtD[:, k, :] if k < 3 else xtA[:, 0, :]

        r14_buf = xtA[:, 1, 0:N]
        r15_buf = xtA[:, 1, N : 2 * N]

        # act_done gates, ACT emission order: acc epis 0..2 (8 each), row-13
        # epi (4), mv epis 6..12 (2 each), mv epi 14, early out, mv epi 15
        actd_acc = {a: 8 * (a + 1) for a in range(NACT)}
        ACTD_R13 = 28
        actd_mv = {u: 28 + 2 * (u - 5) for u in range(6, 13)}  # 30..42
        actd_mv[14] = 44
        actd_mv[15] = 46
        ACT_TOTAL = actd_mv[15]

        def out_dma(eng, sem, is_std, u0, w):
            dst = bass.AP(
                tensor=out_ext,
                offset=is_std * C + u0,
                ap=[[2 * C, B_LOCAL], [RPL, P // B_LOCAL], [1, w]],
            )
            with nc.allow_non_contiguous_dma(reason="4B-per-lane stat column"):
                eng.dma_start(out=dst, in_=MS[:, is_std, u0 : u0 + w]).then_inc(
                    sem, 16
                )

        @block.sync
        def _(sync):
            def pair_src(s0):  # slots (s0, s0+1)
                return xl[:, s0 * N : (s0 + 2) * N]

            # interleave ACT and DVE pair-slabs
            sync.dma_start(out=xtA[:, 0, :], in_=pair_src(0)).then_inc(dma_a[0], 16)
            sync.dma_start(out=dbuf(0), in_=pair_src(6)).then_inc(dma_d[0], 16)
            sync.dma_start(out=xtA[:, 1, :], in_=pair_src(2)).then_inc(dma_a[1], 16)
            sync.dma_start(out=dbuf(1), in_=pair_src(8)).then_inc(dma_d[1], 16)
            sync.dma_start(out=xtA[:, 2, :], in_=pair_src(4)).then_inc(dma_a[2], 16)
            sync.dma_start(out=dbuf(2), in_=pair_src(10)).then_inc(dma_d[2], 16)
            # d3 reuses ACT slab 0's buffer; ACT finished reading it at
            # act_stats >= 4
            sync.wait_ge(act_stats, ACTS[0])
            sync.dma_start(out=dbuf(3), in_=pair_src(12)).then_inc(dma_d[3], 16)
            # rows 14/15 reuse ACT slab 1's buffer
            sync.wait_ge(act_stats, ACTS[1])
            sync.dma_start(out=r14_buf, in_=xl[:, 14 * N : 15 * N]).then_inc(
                dma_r14, 16
            )
            c0 = 0
            for i, w in enumerate(R15_CHUNKS):
                sync.dma_start(
                    out=r15_buf[:, c0 : c0 + w],
                    in_=xl[:, 15 * N + c0 : 15 * N + c0 + w],
                ).then_inc(dma_rc[i], 16)
                c0 += w
            # final std out for slot 15, parallel with scalar's mean out
            sync.wait_ge(act_done, ACT_TOTAL)
            out_dma(sync, fin_sem, 1, RPL - 1, 1)
            sync.wait_ge(out_sem, 16 * 3)
            sync.wait_ge(fin_sem, 16)

        @block.vector
        def _(vector):
            ndve = 0
            nmv = 0

            vector.memset(warm[:, :], 0.0).then_inc(warm_done, 1)

            for k in range(NDVE):
                rows = (0, 1) if k < 3 else (0,)  # d3 row 1 goes to ACT
                vector.wait_ge(dma_d[k], 16)
                buf = dbuf(k)
                for r in rows:
                    si = 2 * k + r  # stats index for slot u = 6+2k+r
                    for g in range(8):
                        vector.bn_stats(
                            out=stats[:, si, g, :],
                            in_=buf[:, (r * 8 + g) * 512 : (r * 8 + g + 1) * 512],
                        ).then_inc(dve_stats, 1)
                        ndve += 1
                assert ndve == D_PAIR[k]
                vector.wait_ge(dve_stats, ndve)
                for r in rows:
                    si = 2 * k + r
                    vector.bn_aggr(
                        out=mv[:, si, :], in_=stats[:, si, 0:8, :]
                    ).then_inc(mv_ready, 1)
                    nmv += 1
                assert nmv == MV[6 + 2 * k + rows[-1]]

            # row 14: one DMA, 8 groups
            vector.wait_ge(dma_r14, 16)
            for g in range(8):
                vector.bn_stats(
                    out=stats[:, 8, g, :], in_=r14_buf[:, g * 512 : (g + 1) * 512]
                ).then_inc(dve_stats, 1)
                ndve += 1
            assert ndve == D_R14
            vector.wait_ge(dve_stats, ndve)
            vector.bn_aggr(out=mv[:, 8, :], in_=stats[:, 8, 0:8, :]).then_inc(
                mv_ready, 1
            )
            nmv += 1
            assert nmv == MV[14]

            # row 15: chunk-paced
            gi = 0
            c0 = 0
            for i, w in enumerate(R15_CHUNKS):
                vector.wait_ge(dma_rc[i], 16)
                gg = c0
                while gg < c0 + w:
                    gw = GROUP15[gi]
                    vector.bn_stats(
                        out=stats[:, 9, gi, :], in_=r15_buf[:, gg : gg + gw]
                    ).then_inc(dve_stats, 1)
                    ndve += 1
                    gg += gw
                    gi += 1
                c0 += w
            assert gi == len(GROUP15) and ndve == DVE_TOTAL
            vector.wait_ge(dve_stats, ndve)
            vector.bn_aggr(
                out=mv[:, 9, :], in_=stats[:, 9, 0 : len(GROUP15), :]
            ).then_inc(mv_ready, 1)
            nmv += 1
            assert nmv == MV[15]

        @block.scalar
        def _(scalar):
            A = 1.0 / np.sqrt(float(N) * (N - 1))
            cact = 0
            nacc = 0

            scalar.wait_ge(warm_done, 1)
            scalar.activation(
                out=warm[:, 0:1],
                in_=warm[:, 1:2],
                func=mybir.ActivationFunctionType.Copy,
            )

            def acc_pass(a):
                nonlocal nacc
                scalar.wait_ge(dma_a[a], 16)
                for r in range(2):
                    row = xtA[:, a, r * N : (r + 1) * N]
                    scalar.activation(
                        out=row,
                        in_=row,
                        func=mybir.ActivationFunctionType.Copy,
                        accum_out=acc[:, a, r, 0:1],
                    ).then_inc(act_stats, 1)
                    nacc += 1
                    scalar.wait_ge(act_stats, nacc)
                    scalar.activation(
                        out=row,
                        in_=row,
                        func=mybir.ActivationFunctionType.Square,
                        accum_out=acc[:, a, r, 1:2],
                    ).then_inc(act_stats, 1)
                    nacc += 1
                assert nacc == ACTS[a]

            def mdst(u):
                return MS[:, 0, u : u + 1]

            def sdst(u):
                return MS[:, 1, u : u + 1]

            def epi_acc(a):
                nonlocal cact
                scalar.wait_ge(act_stats, ACTS[a])
                for r in range(2):
                    u = 2 * a + r
                    scalar.activation(
                        out=mdst(u),
                        in_=acc[:, a, r, 0:1],
                        func=mybir.ActivationFunctionType.Copy,
                        scale=1.0 / N,
                    ).then_inc(act_done, 1)
                    scalar.activation(
                        out=acc[:, a, r, 2:3],
                        in_=acc[:, a, r, 0:1],
                        func=mybir.ActivationFunctionType.Square,
                        scale=A,
                    ).then_inc(act_done, 1)
                    cact += 2
                    scalar.wait_ge(act_done, cact)
                    scalar.activation(
                        out=acc[:, a, r, 2:3],
                        in_=acc[:, a, r, 2:3],
                        func=mybir.ActivationFunctionType.Copy,
                        scale=-1.0,
                    ).then_inc(act_done, 1)
                    cact += 1
                    scalar.wait_ge(act_done, cact)
                    scalar.activation(
                        out=sdst(u),
                        in_=acc[:, a, r, 1:2],
                        func=mybir.ActivationFunctionType.Sqrt,
                        scale=1.0 / (N - 1),
                        bias=acc[:, a, r, 2:3],
                    ).then_inc(act_done, 1)
                    cact += 1
                assert cact == actd_acc[a]

            def epi_mv(u):
                nonlocal cact
                scalar.wait_ge(mv_ready, MV[u])
                si = u - 6
                scalar.copy(out=mdst(u), in_=mv[:, si, 0:1]).then_inc(act_done, 1)
                scalar.activation(
                    out=sdst(u),
                    in_=mv[:, si, 1:2],
                    func=mybir.ActivationFunctionType.Sqrt,
                    scale=float(N) / (N - 1),
                ).then_inc(act_done, 1)
                cact += 2
                assert cact == actd_mv[u]

            def acc_pass_r13():
                # consume row 13 (second half of the d3 pair buffer)
                nonlocal nacc
                scalar.wait_ge(dma_d[3], 16)
                row = dbuf(3)[:, N : 2 * N]
                scalar.activation(
                    out=row,
                    in_=row,
                    func=mybir.ActivationFunctionType.Copy,
                    accum_out=acc[:, 3, 0, 0:1],
                ).then_inc(act_stats, 1)
                nacc += 1
                scalar.wait_ge(act_stats, nacc)
                scalar.activation(
                    out=row,
                    in_=row,
                    func=mybir.ActivationFunctionType.Square,
                    accum_out=acc[:, 3, 0, 1:2],
                ).then_inc(act_stats, 1)
                nacc += 1
                assert nacc == ACTS_R13

            def epi_acc_r13():
                nonlocal cact
                scalar.wait_ge(act_stats, ACTS_R13)
                scalar.activation(
                    out=mdst(13),
                    in_=acc[:, 3, 0, 0:1],
                    func=mybir.ActivationFunctionType.Copy,
                    scale=1.0 / N,
                ).then_inc(act_done, 1)
                scalar.activation(
                    out=acc[:, 3, 0, 2:3],
                    in_=acc[:, 3, 0, 0:1],
                    func=mybir.ActivationFunctionType.Square,
                    scale=A,
                ).then_inc(act_done, 1)
                cact += 2
                scalar.wait_ge(act_done, cact)
                scalar.activation(
                    out=acc[:, 3, 0, 2:3],
                    in_=acc[:, 3, 0, 2:3],
                    func=mybir.ActivationFunctionType.Copy,
                    scale=-1.0,
                ).then_inc(act_done, 1)
                cact += 1
                scalar.wait_ge(act_done, cact)
                scalar.activation(
                    out=sdst(13),
                    in_=acc[:, 3, 0, 1:2],
                    func=mybir.ActivationFunctionType.Sqrt,
                    scale=1.0 / (N - 1),
                    bias=acc[:, 3, 0, 2:3],
                ).then_inc(act_done, 1)
                cact += 1
                assert cact == ACTD_R13

            acc_pass(0)
            acc_pass(1)
            epi_acc(0)
            acc_pass(2)
            epi_acc(1)
            epi_acc(2)
            acc_pass_r13()
            epi_acc_r13()
            for u in range(6, 13):
                epi_mv(u)
            epi_mv(14)
            # early out: mean+std for slots 0..14 (hidden mid-stream)
            scalar.wait_ge(act_done, actd_mv[14])
            out_dma(scalar, out_sem, 0, 0, RPL - 1)
            out_dma(scalar, out_sem, 1, 0, RPL - 1)
            epi_mv(15)
            scalar.wait_ge(act_done, ACT_TOTAL)
            out_dma(scalar, out_sem, 0, RPL - 1, 1)

    return nc


def kernel(f_vol: np.ndarray) -> np.ndarray:
    from concourse.bass_utils import run_bass_kernel_spmd

    if "nc" not in _CACHE:
        _CACHE["nc"] = _build()
    nc = _CACHE["nc"]

    f_vol = np.ascontiguousarray(f_vol, dtype=np.float32)
    in_maps = [
        {"f_vol": f_vol[i * B_LOCAL : (i + 1) * B_LOCAL]} for i in range(N_CORES)
    ]
    res = run_bass_kernel_spmd(nc, in_maps, core_ids=list(range(N_CORES)))
    return np.concatenate([res.results[i]["out"] for i in range(N_CORES)], axis=0)
